# revision 8
# baseline (speedup 1.0000x reference)
"""Trainium2 Bass kernel for nn_EquivariantUpsampleConv.

Key algebraic reduction: the reference nearest-neighbour-upsamples by 4x and
then does a 2x2 windowed equivariant conv + tensor product per HR pixel.
Within each 4x4 block of HR pixels there are only FOUR distinct outputs:
  int : interior     ctx = a@(K0+K1+K2+K3)
  r   : right edge   ctx = a@(K0+K2) + b@(K1+K3)
  btm : bottom edge  ctx = a@(K0+K1) + c@(K2+K3)
  crn : corner       ctx = a@K0 + b@K1 + c@K2 + d@K3
where a = feat[h, w], b = feat[h, w+1], c = feat[h+1, w], d = feat[h+1, w+1]
(edge-clamped) and Kj are the per-neighbour 22x22 matrices obtained by folding
the fixed SH kernel into the weighted aggregation CG tensor.  Each case output
is out = einsum(a_i, Wtp[i,j,k], ctx_j) + a.  The 4x4 HR block is then
  rows 0..2: [int int int r] x256 ; row 3: [btm btm btm crn] x256.

Per-core work is a contiguous band of 32 coarse rows (128 HR rows); the
sharding is pure data parallelism with the +1 row halo materialised host-side.
"""

import numpy as np
from contextlib import ExitStack

C = 22
PAIRS = 484
CHUNKS = [(0, 110), (110, 110), (220, 110), (330, 110), (440, 44)]
NCORES = 8
HH = WW = 256
RPC = HH // NCORES        # coarse rows per core = 32
ITERS = RPC // 2          # two coarse rows per iteration
USE_F32R = True

_CACHE = {}


def _constants(cg_agg, cg_tp, w_agg, w_tp, sh_kernel):
    Wagg = np.einsum('p,pisk->isk', w_agg, cg_agg).astype(np.float64)
    Wtp = np.einsum('p,pijk->ijk', w_tp, cg_tp).astype(np.float64)
    Kj = np.einsum('isk,js->jik', Wagg, sh_kernel.astype(np.float64))
    K0, K1, K2, K3 = Kj
    # ctx matmul weights, order: (block, moving) =
    # (int,a) (r,a) (r,b) (crn,a) (crn,b) (crn,c) (crn,d) (btm,a) (btm,c)
    ctxw = np.stack([
        K0 + K1 + K2 + K3, K0 + K2, K1 + K3,
        K0, K1, K2, K3, K0 + K1, K2 + K3,
    ])  # (9, 22in, 22out)
    CTXW = np.ascontiguousarray(ctxw.transpose(1, 0, 2)).astype(np.float32)
    SELA = np.zeros((C, 5, 110), np.float32)
    SELC = np.zeros((C, 5, 110), np.float32)
    WT = np.zeros((110, 5, C), np.float32)
    wtp_flat = Wtp.reshape(PAIRS, C).astype(np.float32)
    for k, (q0, sz) in enumerate(CHUNKS):
        for p in range(sz):
            q = q0 + p
            SELA[q // C, k, p] = 1.0
            SELC[q % C, k, p] = 1.0
        WT[0:sz, k, :] = wtp_flat[q0:q0 + sz]
    I22 = np.eye(C, dtype=np.float32)
    return CTXW, SELA, SELC, WT, I22


def _build_nc(iters=ITERS):
    import concourse.bacc as bacc
    import concourse.tile as tile
    import concourse.mybir as mybir

    F32 = mybir.dt.float32
    F32R = mybir.dt.float32r
    ZDT = F32R if USE_F32R else F32
    rr = (lambda ap: ap.bitcast(F32R)) if USE_F32R else (lambda ap: ap)

    nrows = 2 * iters
    nc = bacc.Bacc("TRN2", debug=False)
    featT = nc.dram_tensor("featT", (C, nrows + 1, WW + 1), F32,
                           kind="ExternalInput").ap()
    CTXW = nc.dram_tensor("CTXW", (C, 9, C), F32, kind="ExternalInput").ap()
    SELA = nc.dram_tensor("SELA", (C, 5, 110), F32, kind="ExternalInput").ap()
    SELC = nc.dram_tensor("SELC", (C, 5, 110), F32, kind="ExternalInput").ap()
    WT = nc.dram_tensor("WT", (110, 5, C), F32, kind="ExternalInput").ap()
    I22 = nc.dram_tensor("I22", (C, C), F32, kind="ExternalInput").ap()
    OUT = nc.dram_tensor("OUT", (nrows * 4 * WW * 4, C), F32,
                         kind="ExternalOutput").ap()
    # view: HR row = rc*4 + dh ; HR col = pp*4 + dw ; the trailing (dw c)
    # pair is one contiguous 88-element span per coarse pixel
    OUTv = OUT.rearrange("(rc dh pp dw) c -> rc pp dh (dw c)",
                         dh=4, pp=WW, dw=4)

    with tile.TileContext(nc) as tc:
        with ExitStack() as ctx:
            const = ctx.enter_context(tc.tile_pool(name="const", bufs=1))
            feats = ctx.enter_context(tc.tile_pool(name="feats", bufs=2))
            sb = ctx.enter_context(tc.tile_pool(name="sb", bufs=2))
            zpool = ctx.enter_context(tc.tile_pool(name="zpool", bufs=3))
            pmsb = ctx.enter_context(tc.tile_pool(name="pmsb", bufs=2))
            ctxps = ctx.enter_context(tc.tile_pool(name="ctxps", bufs=2, space="PSUM"))
            arps = ctx.enter_context(tc.tile_pool(name="arps", bufs=1, space="PSUM"))
            crps = ctx.enter_context(tc.tile_pool(name="crps", bufs=2, space="PSUM"))
            outps = ctx.enter_context(tc.tile_pool(name="outps", bufs=2, space="PSUM"))
            pmps = ctx.enter_context(tc.tile_pool(name="pmps", bufs=1, space="PSUM"))

            ctxw = const.tile([C, 9, C], ZDT)
            sela = const.tile([C, 5, 110], ZDT)
            selc = const.tile([C, 5, 110], ZDT)
            wt = const.tile([110, 5, C], ZDT)
            i22 = const.tile([C, C], F32)
            nc.sync.dma_start(ctxw[:], rr(CTXW))
            nc.sync.dma_start(sela[:], rr(SELA))
            nc.sync.dma_start(selc[:], rr(SELC))
            nc.sync.dma_start(wt[:], rr(WT))
            nc.sync.dma_start(i22[:], I22)

            for it in range(iters):
                r0 = 2 * it
                # ---- load 3 coarse rows (2 compute + 1 halo), 257 cols
                F = feats.tile([C, 3, WW + 1], ZDT)
                nc.sync.dma_start(F[:], rr(featT[:, r0:r0 + 3, :]))
                a = F[:, 0:2, 0:WW]
                b = F[:, 0:2, 1:WW + 1]
                cc_ = F[:, 1:3, 0:WW]
                d = F[:, 1:3, 1:WW + 1]
                NPX = 2 * WW  # 512 pixels per iteration

                # ---- ctx per case -> own PSUM tile at partition base 0
                # (fp32r matmuls require dst partition base 0), then SBUF
                # case order: 0=int 1=r 2=crn 3=btm
                ctx_sb = sb.tile([C, 4, NPX], ZDT)
                case_plan = [
                    [(a, 0)],
                    [(a, 1), (b, 2)],
                    [(a, 3), (b, 4), (cc_, 5), (d, 6)],
                    [(a, 7), (cc_, 8)],
                ]
                for case, terms in enumerate(case_plan):
                    ctx_ps = ctxps.tile([C, NPX], F32, tag="ctxps")
                    for ti, (mv, wi) in enumerate(terms):
                        nc.tensor.matmul(
                            ctx_ps[:], ctxw[:, wi, :], mv,
                            start=(ti == 0), stop=(ti == len(terms) - 1),
                        )
                    # natively f32r tile: ACT write rounds -> legal f32r input
                    nc.scalar.copy(ctx_sb[:, case, :], ctx_ps[:])

                # ---- A_rep: broadcast a_i to pair rows, per chunk
                ar_sb = sb.tile([110, 5, NPX], F32)
                for k, (q0, sz) in enumerate(CHUNKS):
                    ar_ps = arps.tile([128, NPX], F32, tag="arps")
                    nc.tensor.matmul(ar_ps[0:sz, :], sela[:, k, 0:sz], a,
                                     start=True, stop=True)
                    nc.scalar.copy(ar_sb[0:sz, k, :], ar_ps[0:sz, :])

                # ---- per case: C_rep bcast, z mult, TP contraction, residual
                res_sb = sb.tile([C, 4, NPX], F32)
                av = a.bitcast(F32) if USE_F32R else a
                for case in range(4):
                    out_ps = outps.tile([C, NPX], F32, tag="outps")
                    for k, (q0, sz) in enumerate(CHUNKS):
                        cr_ps = crps.tile([128, NPX], F32, tag="crps")
                        nc.tensor.matmul(
                            cr_ps[0:sz, :], selc[:, k, 0:sz],
                            ctx_sb[:, case, :],
                            start=True, stop=True,
                        )
                        z = zpool.tile([128, NPX], ZDT, tag="z")
                        nc.vector.tensor_mul(z[0:sz, :], ar_sb[0:sz, k, :],
                                             cr_ps[0:sz, :])
                        nc.tensor.matmul(
                            out_ps[:], wt[0:sz, k, :], z[0:sz, :],
                            start=(k == 0), stop=(k == 4),
                        )
                    # residual add (out = TP + a), cases along free dim
                    nc.vector.tensor_add(
                        res_sb[:, case, :].rearrange("c (t x) -> c t x", t=2),
                        out_ps[:].rearrange("c (t x) -> c t x", t=2),
                        av,
                    )

                # ---- transpose to pixel-major: pm[p, t, case*22+cc]
                pm_ps = pmps.tile([128, 4, 4 * C], F32)
                for t in range(4):
                    for case in range(4):
                        nc.tensor.transpose(
                            pm_ps[:, t, C * case:C * case + C],
                            res_sb[:, case, 128 * t:128 * (t + 1)],
                            i22[:],
                        )
                # ---- build interleaved HR-row panes on ACT:
                # pmA = [int int int r] ; pmB = [btm btm btm crn]  (88 el/px4)
                pmA = pmsb.tile([128, 4, 4 * C], F32, tag="pmA")
                pmB = pmsb.tile([128, 4, 4 * C], F32, tag="pmB")
                nc.scalar.copy(
                    pmA[:].rearrange("p t (dw c) -> p t dw c", dw=4)[:, :, 0:3, :],
                    pm_ps[:, :, None, 0:C].to_broadcast((128, 4, 3, C)),
                )
                nc.scalar.copy(
                    pmA[:].rearrange("p t (dw c) -> p t dw c", dw=4)[:, :, 3, :],
                    pm_ps[:, :, C:2 * C],
                )
                nc.scalar.copy(
                    pmB[:].rearrange("p t (dw c) -> p t dw c", dw=4)[:, :, 0:3, :],
                    pm_ps[:, :, None, 3 * C:4 * C].to_broadcast((128, 4, 3, C)),
                )
                nc.scalar.copy(
                    pmB[:].rearrange("p t (dw c) -> p t dw c", dw=4)[:, :, 3, :],
                    pm_ps[:, :, 2 * C:3 * C],
                )

                # ---- stores with DMA row replication (3x for rowA)
                for lr in range(2):          # local coarse row
                    rc = r0 + lr
                    for tt in range(2):      # 128-pixel half of the row
                        t = 2 * lr + tt
                        ps = slice(128 * tt, 128 * (tt + 1))
                        nc.sync.dma_start(
                            OUTv[rc, ps, 0:3, :],
                            pmA[:, t, None, :].to_broadcast((128, 3, 4 * C)),
                        )
                        nc.sync.dma_start(OUTv[rc, ps, 3, :], pmB[:, t, :])
    nc.compile()
    return nc


def _prep_inputs(f4, f6, sh_kernel, cg_agg, cg_tp, w_agg, w_tp):
    f4 = np.asarray(f4, dtype=np.float32)
    f6 = np.asarray(f6, dtype=np.float32)
    feat = np.concatenate([f4, f6], axis=-1).reshape(HH, WW, C)
    fp = np.pad(feat, ((0, 1), (0, 1), (0, 0)), mode='edge')  # (257,257,22)
    featT = np.ascontiguousarray(fp.transpose(2, 0, 1))       # (22,257,257)
    CTXW, SELA, SELC, WT, I22 = _constants(
        np.asarray(cg_agg, np.float32), np.asarray(cg_tp, np.float32),
        np.asarray(w_agg, np.float32), np.asarray(w_tp, np.float32),
        np.asarray(sh_kernel, np.float32))
    in_maps = []
    for m in range(NCORES):
        sl = np.ascontiguousarray(featT[:, 32 * m:32 * m + 33, :])
        in_maps.append(dict(featT=sl, CTXW=CTXW, SELA=SELA, SELC=SELC,
                            WT=WT, I22=I22))
    return in_maps


def kernel(f4, f6, sh_kernel, cg_agg, cg_tp, w_agg, w_tp, H, W, _trace=False):
    assert int(H) == HH and int(W) == WW
    from concourse.bass_utils import run_bass_kernel_spmd

    if "nc" not in _CACHE:
        _CACHE["nc"] = _build_nc()
    nc = _CACHE["nc"]
    in_maps = _prep_inputs(f4, f6, sh_kernel, cg_agg, cg_tp, w_agg, w_tp)
    try:
        res = run_bass_kernel_spmd(nc, in_maps, list(range(NCORES)),
                                   trace=_trace)
    except ModuleNotFoundError:
        # NTFF profiling hook unavailable in this environment
        res = run_bass_kernel_spmd(nc, in_maps, list(range(NCORES)),
                                   trace=False)
    out = np.concatenate([res.results[m]["OUT"] for m in range(NCORES)], axis=0)
    if _trace:
        _CACHE["last_exec_time_ns"] = res.exec_time_ns
        _CACHE["last_profile"] = res
    return (np.ascontiguousarray(out[:, :9]),
            np.ascontiguousarray(out[:, 9:]))


# revision 16
# speedup vs baseline: 1.4259x; 1.4259x over previous
"""Trainium2 Bass kernel for nn_EquivariantUpsampleConv.

Key algebraic reduction: the reference nearest-neighbour-upsamples by 4x and
then does a 2x2 windowed equivariant conv + tensor product per HR pixel.
Within each 4x4 block of HR pixels there are only FOUR distinct outputs:
  int : interior     ctx = a@(K0+K1+K2+K3)
  r   : right edge   ctx = a@(K0+K2) + b@(K1+K3)
  btm : bottom edge  ctx = a@(K0+K1) + c@(K2+K3)
  crn : corner       ctx = a@K0 + b@K1 + c@K2 + d@K3
where a = feat[h, w], b = feat[h, w+1], c = feat[h+1, w], d = feat[h+1, w+1]
(edge-clamped) and Kj are the per-neighbour 22x22 matrices obtained by folding
the fixed SH kernel into the weighted aggregation CG tensor.  Each case output
is out = einsum(a_i, Wtp[i,j,k], ctx_j) + a.  The 4x4 HR block is then
  rows 0..2: [int int int r] x256 ; row 3: [btm btm btm crn] x256.

Per-core work is a contiguous band of 32 coarse rows (128 HR rows); the
sharding is pure data parallelism with the +1 row halo materialised host-side.
"""

import numpy as np
from contextlib import ExitStack

C = 22
PAIRS = 484
CHUNKS = [(0, 121), (121, 121), (242, 121), (363, 121)]
NCORES = 8
HH = WW = 256
RPC = HH // NCORES        # coarse rows per core = 32
ITERS = RPC // 2          # two coarse rows per iteration
USE_F32R = True

# maximal same-i runs of pair index q = i*22 + j within each 121-chunk:
# (chunk, row_lo, row_hi, i)
AR_RUNS = []
for _m, (_q0, _sz) in enumerate(CHUNKS):
    _q = _q0
    while _q < _q0 + _sz:
        _i = _q // C
        _hi = min((_i + 1) * C, _q0 + _sz)
        AR_RUNS.append((_m, _q - _q0, _hi - _q0, _i))
        _q = _hi

_CACHE = {}


def _constants(cg_agg, cg_tp, w_agg, w_tp, sh_kernel):
    Wagg = np.einsum('p,pisk->isk', w_agg, cg_agg).astype(np.float64)
    Wtp = np.einsum('p,pijk->ijk', w_tp, cg_tp).astype(np.float64)
    Kj = np.einsum('isk,js->jik', Wagg, sh_kernel.astype(np.float64))
    K0, K1, K2, K3 = Kj
    # packed ctx weights: 4 accumulating matmuls (moving a, b, c, d), each
    # M=88 over case blocks [int | r | crn | btm]; zero blocks where a
    # neighbour does not contribute
    # case blocks padded to 32-aligned partition offsets (PSUM reads must
    # start at 0/32/64/96): M = 118, blocks at 0 / 32 / 64 / 96
    Z = np.zeros((C, C))
    G = np.zeros((C, 10))
    CTXW = np.stack([
        np.concatenate([K0 + K1 + K2 + K3, G, K0 + K2, G, K0, G, K0 + K1], 1),
        np.concatenate([Z, G, K1 + K3, G, K1, G, Z], 1),               # b
        np.concatenate([Z, G, Z, G, K2, G, K2 + K3], 1),               # c
        np.concatenate([Z, G, Z, G, K3, G, Z], 1),                     # d
    ]).transpose(1, 0, 2)                                  # (22, 4, 118)
    CTXW = np.ascontiguousarray(CTXW).astype(np.float32)
    SELA = np.zeros((C, 4, 121), np.float32)
    SELC = np.zeros((C, 4, 121), np.float32)
    WT = np.zeros((121, 4, C), np.float32)
    wtp_flat = Wtp.reshape(PAIRS, C).astype(np.float32)
    for k, (q0, sz) in enumerate(CHUNKS):
        for p in range(sz):
            SELA[(q0 + p) // C, k, p] = 1.0
            SELC[(q0 + p) % C, k, p] = 1.0
        WT[0:sz, k, :] = wtp_flat[q0:q0 + sz]
    I22 = np.eye(C, dtype=np.float32)
    return CTXW, SELA, SELC, WT, I22


def _build_nc(iters=ITERS):
    import concourse.bacc as bacc
    import concourse.tile as tile
    import concourse.mybir as mybir

    F32 = mybir.dt.float32
    F32R = mybir.dt.float32r
    ZDT = F32R if USE_F32R else F32
    rr = (lambda ap: ap.bitcast(F32R)) if USE_F32R else (lambda ap: ap)

    nrows = 2 * iters
    nc = bacc.Bacc("TRN2", debug=False)
    featT = nc.dram_tensor("featT", (C, nrows + 1, WW + 1), F32,
                           kind="ExternalInput").ap()
    CTXW = nc.dram_tensor("CTXW", (C, 4, 118), F32, kind="ExternalInput").ap()
    SELA = nc.dram_tensor("SELA", (C, 4, 121), F32, kind="ExternalInput").ap()
    SELC = nc.dram_tensor("SELC", (C, 4, 121), F32, kind="ExternalInput").ap()
    WT = nc.dram_tensor("WT", (121, 4, C), F32, kind="ExternalInput").ap()
    I22 = nc.dram_tensor("I22", (C, C), F32, kind="ExternalInput").ap()
    OUT = nc.dram_tensor("OUT", (nrows * 4 * WW * 4, C), F32,
                         kind="ExternalOutput").ap()
    # view: HR row = rc*4 + dh ; HR col = pp*4 + dw ; the trailing (dw c)
    # pair is one contiguous 88-element span per coarse pixel
    OUTv = OUT.rearrange("(rc dh pp dw) c -> rc pp dh (dw c)",
                         dh=4, pp=WW, dw=4)

    with tile.TileContext(nc) as tc:
        with ExitStack() as ctx:
            const = ctx.enter_context(tc.tile_pool(name="const", bufs=1))
            feats = ctx.enter_context(tc.tile_pool(name="feats", bufs=2))
            sb = ctx.enter_context(tc.tile_pool(name="sb", bufs=2))
            zpool = ctx.enter_context(tc.tile_pool(name="zpool", bufs=4))
            pmsb = ctx.enter_context(tc.tile_pool(name="pmsb", bufs=2))
            ctxps = ctx.enter_context(tc.tile_pool(name="ctxps", bufs=1, space="PSUM"))
            arps = ctx.enter_context(tc.tile_pool(name="arps", bufs=1, space="PSUM"))
            crps = ctx.enter_context(tc.tile_pool(name="crps", bufs=3, space="PSUM"))
            outps = ctx.enter_context(tc.tile_pool(name="outps", bufs=2, space="PSUM"))
            pmps = ctx.enter_context(tc.tile_pool(name="pmps", bufs=1, space="PSUM"))

            ctxw = const.tile([C, 4, 118], ZDT)
            sela = const.tile([C, 4, 121], ZDT)
            selc = const.tile([C, 4, 121], ZDT)
            wt = const.tile([121, 4, C], ZDT)
            i22 = const.tile([C, C], ZDT)
            nc.sync.dma_start(ctxw[:], rr(CTXW))
            nc.sync.dma_start(sela[:], rr(SELA))
            nc.sync.dma_start(selc[:], rr(SELC))
            nc.sync.dma_start(wt[:], rr(WT))
            nc.sync.dma_start(i22[:], rr(I22))

            for it in range(iters):
                r0 = 2 * it
                # ---- load 3 coarse rows (2 compute + 1 halo), 257 cols
                F = feats.tile([C, 3, WW + 1], ZDT)
                nc.sync.dma_start(F[:], rr(featT[:, r0:r0 + 3, :]))
                a = F[:, 0:2, 0:WW]
                b = F[:, 0:2, 1:WW + 1]
                cc_ = F[:, 1:3, 0:WW]
                d = F[:, 1:3, 1:WW + 1]
                NPX = 2 * WW  # 512 pixels per iteration

                # ---- ctx: 4 accumulating matmuls, all cases in one
                # [88, 512] PSUM tile at base 0 (zero-padded block weights)
                # case order: 0=int 1=r 2=crn 3=btm
                ctx_ps = ctxps.tile([118, NPX], F32, tag="ctxps")
                for ti, mv in enumerate((a, b, cc_, d)):
                    nc.tensor.matmul(ctx_ps[:], ctxw[:, ti, :], mv,
                                     start=(ti == 0), stop=(ti == 3))
                # natively f32r tile: ACT write rounds -> legal f32r input
                ctx_sb = sb.tile([C, 4, NPX], ZDT)
                for case in range(4):
                    nc.scalar.copy(ctx_sb[:, case, :],
                                   ctx_ps[32 * case:32 * case + C, :])

                # ---- A_rep: PE broadcast of a_i to pair rows, per chunk
                ar_sb = sb.tile([121, 4, NPX], F32)
                for k in range(4):
                    ar_ps = arps.tile([121, NPX], F32, tag="arps")
                    nc.tensor.matmul(ar_ps[:], sela[:, k, :], a,
                                     start=True, stop=True)
                    nc.scalar.copy(ar_sb[:, k, :], ar_ps[:])

                # ---- per case: C_rep bcast -> z mult -> TP contraction.
                # Odd chunks take an ACT hop to SBUF so their z-mult runs in
                # DVE 2x mode; the residual is folded into the TP as a 5th
                # identity-weighted accumulation pass, and ACT copies the
                # finished case to SBUF for the transposes.
                res_sb = sb.tile([C, 4, NPX], ZDT)
                for case in range(4):
                    out_ps = outps.tile([C, NPX], F32, tag="outps")
                    for k in range(4):
                        cr_ps = crps.tile([121, NPX], F32, tag="crps")
                        nc.tensor.matmul(
                            cr_ps[:], selc[:, k, :], ctx_sb[:, case, :],
                            start=True, stop=True,
                        )
                        z = zpool.tile([121, NPX], ZDT, tag="z")
                        nc.vector.tensor_mul(z[:], ar_sb[:, k, :], cr_ps[:])
                        nc.tensor.matmul(
                            out_ps[:], wt[:, k, :], z[:],
                            start=(k == 0), stop=False,
                        )
                    # residual: out += I22 @ a as the closing accumulation
                    nc.tensor.matmul(out_ps[:], i22[:], a,
                                     start=False, stop=True)
                    nc.scalar.copy(res_sb[:, case, :], out_ps[:])

                # ---- transpose to pixel-major: pm[p, t, case*22+cc]
                pm_ps = pmps.tile([128, 4, 4 * C], ZDT)
                for t in range(4):
                    for case in range(4):
                        nc.tensor.transpose(
                            pm_ps[:, t, C * case:C * case + C],
                            res_sb[:, case, 128 * t:128 * (t + 1)],
                            i22[:],
                        )
                # ---- build interleaved HR-row panes on ACT:
                # pmA = [int int int r] ; pmB = [btm btm btm crn]  (88 el/px4)
                pmA = pmsb.tile([128, 4, 4 * C], F32, tag="pmA")
                pmB = pmsb.tile([128, 4, 4 * C], F32, tag="pmB")
                nc.scalar.copy(
                    pmA[:].rearrange("p t (dw c) -> p t dw c", dw=4)[:, :, 0:3, :],
                    pm_ps[:, :, None, 0:C].to_broadcast((128, 4, 3, C)),
                )
                nc.scalar.copy(
                    pmA[:].rearrange("p t (dw c) -> p t dw c", dw=4)[:, :, 3, :],
                    pm_ps[:, :, C:2 * C],
                )
                nc.scalar.copy(
                    pmB[:].rearrange("p t (dw c) -> p t dw c", dw=4)[:, :, 0:3, :],
                    pm_ps[:, :, None, 3 * C:4 * C].to_broadcast((128, 4, 3, C)),
                )
                nc.scalar.copy(
                    pmB[:].rearrange("p t (dw c) -> p t dw c", dw=4)[:, :, 3, :],
                    pm_ps[:, :, 2 * C:3 * C],
                )

                # ---- stores with DMA row replication (3x for rowA)
                for lr in range(2):          # local coarse row
                    rc = r0 + lr
                    for tt in range(2):      # 128-pixel half of the row
                        t = 2 * lr + tt
                        ps = slice(128 * tt, 128 * (tt + 1))
                        nc.sync.dma_start(
                            OUTv[rc, ps, 0:3, :],
                            pmA[:, t, None, :].to_broadcast((128, 3, 4 * C)),
                        )
                        nc.sync.dma_start(OUTv[rc, ps, 3, :], pmB[:, t, :])
    nc.compile()
    return nc


def _prep_inputs(f4, f6, sh_kernel, cg_agg, cg_tp, w_agg, w_tp):
    f4 = np.asarray(f4, dtype=np.float32)
    f6 = np.asarray(f6, dtype=np.float32)
    feat = np.concatenate([f4, f6], axis=-1).reshape(HH, WW, C)
    fp = np.pad(feat, ((0, 1), (0, 1), (0, 0)), mode='edge')  # (257,257,22)
    featT = np.ascontiguousarray(fp.transpose(2, 0, 1))       # (22,257,257)
    CTXW, SELA, SELC, WT, I22 = _constants(
        np.asarray(cg_agg, np.float32), np.asarray(cg_tp, np.float32),
        np.asarray(w_agg, np.float32), np.asarray(w_tp, np.float32),
        np.asarray(sh_kernel, np.float32))
    in_maps = []
    for m in range(NCORES):
        sl = np.ascontiguousarray(featT[:, 32 * m:32 * m + 33, :])
        in_maps.append(dict(featT=sl, CTXW=CTXW, SELA=SELA, SELC=SELC,
                            WT=WT, I22=I22))
    return in_maps


def kernel(f4, f6, sh_kernel, cg_agg, cg_tp, w_agg, w_tp, H, W, _trace=False):
    assert int(H) == HH and int(W) == WW
    from concourse.bass_utils import run_bass_kernel_spmd

    if "nc" not in _CACHE:
        _CACHE["nc"] = _build_nc()
    nc = _CACHE["nc"]
    in_maps = _prep_inputs(f4, f6, sh_kernel, cg_agg, cg_tp, w_agg, w_tp)
    try:
        res = run_bass_kernel_spmd(nc, in_maps, list(range(NCORES)),
                                   trace=_trace)
    except ModuleNotFoundError:
        # NTFF profiling hook unavailable in this environment
        res = run_bass_kernel_spmd(nc, in_maps, list(range(NCORES)),
                                   trace=False)
    out = np.concatenate([res.results[m]["OUT"] for m in range(NCORES)], axis=0)
    if _trace:
        _CACHE["last_exec_time_ns"] = res.exec_time_ns
        _CACHE["last_profile"] = res
    return (np.ascontiguousarray(out[:, :9]),
            np.ascontiguousarray(out[:, 9:]))


# revision 22
# speedup vs baseline: 1.4324x; 1.0046x over previous
"""Trainium2 Bass kernel for nn_EquivariantUpsampleConv.

Key algebraic reduction: the reference nearest-neighbour-upsamples by 4x and
then does a 2x2 windowed equivariant conv + tensor product per HR pixel.
Within each 4x4 block of HR pixels there are only FOUR distinct outputs:
  int : interior     ctx = a@(K0+K1+K2+K3)
  r   : right edge   ctx = a@(K0+K2) + b@(K1+K3)
  btm : bottom edge  ctx = a@(K0+K1) + c@(K2+K3)
  crn : corner       ctx = a@K0 + b@K1 + c@K2 + d@K3
where a = feat[h, w], b = feat[h, w+1], c = feat[h+1, w], d = feat[h+1, w+1]
(edge-clamped) and Kj are the per-neighbour 22x22 matrices obtained by folding
the fixed SH kernel into the weighted aggregation CG tensor.  Each case output
is out = einsum(a_i, Wtp[i,j,k], ctx_j) + a.  The 4x4 HR block is then
  rows 0..2: [int int int r] x256 ; row 3: [btm btm btm crn] x256.

Per-core work is a contiguous band of 32 coarse rows (128 HR rows); the
sharding is pure data parallelism with the +1 row halo materialised host-side.
"""

import numpy as np
from contextlib import ExitStack

C = 22
PAIRS = 484
CHUNKS = [(0, 121), (121, 121), (242, 121), (363, 121)]
NCORES = 8
HH = WW = 256
RPC = HH // NCORES        # coarse rows per core = 32
ITERS = RPC // 2          # two coarse rows per iteration
USE_F32R = True

# maximal same-i runs of pair index q = i*22 + j within each 121-chunk:
# (chunk, row_lo, row_hi, i)
AR_RUNS = []
for _m, (_q0, _sz) in enumerate(CHUNKS):
    _q = _q0
    while _q < _q0 + _sz:
        _i = _q // C
        _hi = min((_i + 1) * C, _q0 + _sz)
        AR_RUNS.append((_m, _q - _q0, _hi - _q0, _i))
        _q = _hi

_CACHE = {}


def _constants(cg_agg, cg_tp, w_agg, w_tp, sh_kernel):
    Wagg = np.einsum('p,pisk->isk', w_agg, cg_agg).astype(np.float64)
    Wtp = np.einsum('p,pijk->ijk', w_tp, cg_tp).astype(np.float64)
    Kj = np.einsum('isk,js->jik', Wagg, sh_kernel.astype(np.float64))
    K0, K1, K2, K3 = Kj
    # packed ctx weights: 4 accumulating matmuls (moving a, b, c, d), each
    # M=88 over case blocks [int | r | crn | btm]; zero blocks where a
    # neighbour does not contribute
    # case blocks padded to 32-aligned partition offsets (PSUM reads must
    # start at 0/32/64/96): M = 118, blocks at 0 / 32 / 64 / 96
    Z = np.zeros((C, C))
    G = np.zeros((C, 10))
    CTXW = np.stack([
        np.concatenate([K0 + K1 + K2 + K3, G, K0 + K2, G, K0, G, K0 + K1], 1),
        np.concatenate([Z, G, K1 + K3, G, K1, G, Z], 1),               # b
        np.concatenate([Z, G, Z, G, K2, G, K2 + K3], 1),               # c
        np.concatenate([Z, G, Z, G, K3, G, Z], 1),                     # d
    ]).transpose(1, 0, 2)                                  # (22, 4, 118)
    CTXW = np.ascontiguousarray(CTXW).astype(np.float32)
    SELA = np.zeros((C, 4, 121), np.float32)
    SELC = np.zeros((C, 4, 121), np.float32)
    WT = np.zeros((121, 4, 32), np.float32)
    wtp_flat = Wtp.reshape(PAIRS, C).astype(np.float32)
    for k, (q0, sz) in enumerate(CHUNKS):
        for p in range(sz):
            SELA[(q0 + p) // C, k, p] = 1.0
            SELC[(q0 + p) % C, k, p] = 1.0
        WT[0:sz, k, 0:C] = wtp_flat[q0:q0 + sz]
    I54 = np.eye(54, dtype=np.float32)
    return CTXW, SELA, SELC, WT, I54


def _build_nc(iters=ITERS):
    import concourse.bacc as bacc
    import concourse.tile as tile
    import concourse.mybir as mybir

    F32 = mybir.dt.float32
    F32R = mybir.dt.float32r
    ZDT = F32R if USE_F32R else F32
    rr = (lambda ap: ap.bitcast(F32R)) if USE_F32R else (lambda ap: ap)

    nrows = 2 * iters
    nc = bacc.Bacc("TRN2", debug=False)
    featT = nc.dram_tensor("featT", (C, nrows + 1, WW + 1), F32,
                           kind="ExternalInput").ap()
    CTXW = nc.dram_tensor("CTXW", (C, 4, 118), F32, kind="ExternalInput").ap()
    SELA = nc.dram_tensor("SELA", (C, 4, 121), F32, kind="ExternalInput").ap()
    SELC = nc.dram_tensor("SELC", (C, 4, 121), F32, kind="ExternalInput").ap()
    WT = nc.dram_tensor("WT", (121, 4, 32), F32, kind="ExternalInput").ap()
    I54 = nc.dram_tensor("I54", (54, 54), F32, kind="ExternalInput").ap()
    OUT = nc.dram_tensor("OUT", (nrows * 4 * WW * 4, C), F32,
                         kind="ExternalOutput").ap()
    # view: HR row = rc*4 + dh ; HR col = pp*4 + dw ; the trailing (dw c)
    # pair is one contiguous 88-element span per coarse pixel
    OUTv = OUT.rearrange("(rc dh pp dw) c -> rc pp dh (dw c)",
                         dh=4, pp=WW, dw=4)

    with tile.TileContext(nc) as tc:
        with ExitStack() as ctx:
            const = ctx.enter_context(tc.tile_pool(name="const", bufs=1))
            feats = ctx.enter_context(tc.tile_pool(name="feats", bufs=2))
            sb = ctx.enter_context(tc.tile_pool(name="sb", bufs=2))
            zpool = ctx.enter_context(tc.tile_pool(name="zpool", bufs=4))
            pmsb = ctx.enter_context(tc.tile_pool(name="pmsb", bufs=2))
            ctxps = ctx.enter_context(tc.tile_pool(name="ctxps", bufs=1, space="PSUM"))
            arps = ctx.enter_context(tc.tile_pool(name="arps", bufs=1, space="PSUM"))
            crps = ctx.enter_context(tc.tile_pool(name="crps", bufs=3, space="PSUM"))
            outps = ctx.enter_context(tc.tile_pool(name="outps", bufs=2, space="PSUM"))
            pmps = ctx.enter_context(tc.tile_pool(name="pmps", bufs=1, space="PSUM"))

            ctxw = const.tile([C, 4, 118], ZDT)
            sela = const.tile([C, 4, 121], ZDT)
            selc = const.tile([C, 4, 121], ZDT)
            wt = const.tile([121, 4, 32], ZDT)
            i54 = const.tile([54, 54], ZDT)
            nc.sync.dma_start(ctxw[:], rr(CTXW))
            nc.sync.dma_start(sela[:], rr(SELA))
            nc.sync.dma_start(selc[:], rr(SELC))
            nc.sync.dma_start(wt[:], rr(WT))
            nc.sync.dma_start(i54[:], rr(I54))

            for it in range(iters):
                r0 = 2 * it
                # ---- load 3 coarse rows (2 compute + 1 halo), 257 cols
                F = feats.tile([C, 3, WW + 1], ZDT)
                nc.sync.dma_start(F[:], rr(featT[:, r0:r0 + 3, :]))
                a = F[:, 0:2, 0:WW]
                b = F[:, 0:2, 1:WW + 1]
                cc_ = F[:, 1:3, 0:WW]
                d = F[:, 1:3, 1:WW + 1]
                NPX = 2 * WW  # 512 pixels per iteration

                # ---- ctx: 4 accumulating matmuls, all cases in one
                # [88, 512] PSUM tile at base 0 (zero-padded block weights)
                # case order: 0=int 1=r 2=crn 3=btm
                ctx_ps = ctxps.tile([118, NPX], F32, tag="ctxps")
                for ti, mv in enumerate((a, b, cc_, d)):
                    nc.tensor.matmul(ctx_ps[:], ctxw[:, ti, :], mv,
                                     start=(ti == 0), stop=(ti == 3))
                # natively f32r tile: ACT write rounds -> legal f32r input
                ctx_sb = sb.tile([C, 4, NPX], ZDT)
                for case in range(4):
                    nc.scalar.copy(ctx_sb[:, case, :],
                                   ctx_ps[32 * case:32 * case + C, :])

                # ---- A_rep: PE broadcast of a_i to pair rows, per chunk
                ar_sb = sb.tile([121, 4, NPX], F32)
                for k in range(4):
                    ar_ps = arps.tile([121, NPX], F32, tag="arps")
                    nc.tensor.matmul(ar_ps[:], sela[:, k, :], a,
                                     start=True, stop=True)
                    nc.scalar.copy(ar_sb[:, k, :], ar_ps[:])

                # ---- per case: C_rep bcast -> z mult -> TP contraction.
                # Odd chunks take an ACT hop to SBUF so their z-mult runs in
                # DVE 2x mode; the residual is folded into the TP as a 5th
                # identity-weighted accumulation pass, and ACT copies the
                # finished case to SBUF for the transposes.
                res_sb = sb.tile([64, 2, NPX], ZDT)
                for case in range(4):
                    out_ps = outps.tile([32, NPX], F32, tag="outps")
                    for k in range(4):
                        cr_ps = crps.tile([121, NPX], F32, tag="crps")
                        nc.tensor.matmul(
                            cr_ps[:], selc[:, k, :], ctx_sb[:, case, :],
                            start=True, stop=True,
                        )
                        z = zpool.tile([121, NPX], ZDT, tag="z")
                        nc.vector.tensor_mul(z[:], ar_sb[:, k, :], cr_ps[:])
                        nc.tensor.matmul(
                            out_ps[:], wt[:, k, :], z[:],
                            start=(k == 0), stop=False,
                        )
                    # residual: out += I22 @ a as the closing accumulation
                    nc.tensor.matmul(out_ps[:], i54[0:C, 0:32], a,
                                     start=False, stop=True)
                    nc.scalar.copy(
                        res_sb[32 * (case % 2):32 * (case % 2) + 32, case // 2, :],
                        out_ps[:])

                # ---- transpose to pixel-major: pm[p, t, case*22+cc]
                pm_ps = pmps.tile([128, 4, 108], ZDT)
                for t in range(4):
                    for sp in range(2):       # case pair (0,1) / (2,3)
                        nc.tensor.transpose(
                            pm_ps[:, t, 54 * sp:54 * sp + 54],
                            res_sb[0:54, sp, 128 * t:128 * (t + 1)],
                            i54[:],
                        )
                # ---- build interleaved HR-row panes on ACT:
                # pmA = [int int int r] ; pmB = [btm btm btm crn]  (88 el/px4)
                pmA = pmsb.tile([128, 4, 4 * C], F32, tag="pmA")
                pmB = pmsb.tile([128, 4, 4 * C], F32, tag="pmB")
                nc.scalar.copy(
                    pmA[:].rearrange("p t (dw c) -> p t dw c", dw=4)[:, :, 0:3, :],
                    pm_ps[:, :, None, 0:C].to_broadcast((128, 4, 3, C)),
                )
                nc.scalar.copy(
                    pmA[:].rearrange("p t (dw c) -> p t dw c", dw=4)[:, :, 3, :],
                    pm_ps[:, :, 32:32 + C],
                )
                nc.scalar.copy(
                    pmB[:].rearrange("p t (dw c) -> p t dw c", dw=4)[:, :, 0:3, :],
                    pm_ps[:, :, None, 86:86 + C].to_broadcast((128, 4, 3, C)),
                )
                nc.scalar.copy(
                    pmB[:].rearrange("p t (dw c) -> p t dw c", dw=4)[:, :, 3, :],
                    pm_ps[:, :, 54:54 + C],
                )

                # ---- stores with DMA row replication (3x for rowA)
                for lr in range(2):          # local coarse row
                    rc = r0 + lr
                    for tt in range(2):      # 128-pixel half of the row
                        t = 2 * lr + tt
                        ps = slice(128 * tt, 128 * (tt + 1))
                        nc.sync.dma_start(
                            OUTv[rc, ps, 0:3, :],
                            pmA[:, t, None, :].to_broadcast((128, 3, 4 * C)),
                        )
                        nc.sync.dma_start(OUTv[rc, ps, 3, :], pmB[:, t, :])
    nc.compile()
    return nc


def _prep_inputs(f4, f6, sh_kernel, cg_agg, cg_tp, w_agg, w_tp):
    f4 = np.asarray(f4, dtype=np.float32)
    f6 = np.asarray(f6, dtype=np.float32)
    feat = np.concatenate([f4, f6], axis=-1).reshape(HH, WW, C)
    fp = np.pad(feat, ((0, 1), (0, 1), (0, 0)), mode='edge')  # (257,257,22)
    featT = np.ascontiguousarray(fp.transpose(2, 0, 1))       # (22,257,257)
    CTXW, SELA, SELC, WT, I54 = _constants(
        np.asarray(cg_agg, np.float32), np.asarray(cg_tp, np.float32),
        np.asarray(w_agg, np.float32), np.asarray(w_tp, np.float32),
        np.asarray(sh_kernel, np.float32))
    in_maps = []
    for m in range(NCORES):
        sl = np.ascontiguousarray(featT[:, 32 * m:32 * m + 33, :])
        in_maps.append(dict(featT=sl, CTXW=CTXW, SELA=SELA, SELC=SELC,
                            WT=WT, I54=I54))
    return in_maps


def kernel(f4, f6, sh_kernel, cg_agg, cg_tp, w_agg, w_tp, H, W, _trace=False):
    assert int(H) == HH and int(W) == WW
    from concourse.bass_utils import run_bass_kernel_spmd

    if "nc" not in _CACHE:
        _CACHE["nc"] = _build_nc()
    nc = _CACHE["nc"]
    in_maps = _prep_inputs(f4, f6, sh_kernel, cg_agg, cg_tp, w_agg, w_tp)
    try:
        res = run_bass_kernel_spmd(nc, in_maps, list(range(NCORES)),
                                   trace=_trace)
    except ModuleNotFoundError:
        # NTFF profiling hook unavailable in this environment
        res = run_bass_kernel_spmd(nc, in_maps, list(range(NCORES)),
                                   trace=False)
    out = np.concatenate([res.results[m]["OUT"] for m in range(NCORES)], axis=0)
    if _trace:
        _CACHE["last_exec_time_ns"] = res.exec_time_ns
        _CACHE["last_profile"] = res
    return (np.ascontiguousarray(out[:, :9]),
            np.ascontiguousarray(out[:, 9:]))
